# revision 16
# baseline (speedup 1.0000x reference)
"""Differential attention (two-softmax diff + GroupNorm) on 8 TRN2 cores.

Sharding: 16 heads / 8 cores = 2 heads per core (head-parallel, no
collectives). GroupNorm stats are per-(batch, head) so each core is fully
independent.

Device layout choices (host prepares everything):
  - Q, K per head are host-transposed to [128(d), 2048(s)] fp16: partitions
    0-63 hold half-1 (q1/k1), partitions 64-127 hold half-2. QK^T then
    contracts over the partition dim directly, producing transposed score
    blocks S^T[key, query] in PSUM (fp32).
  - V per head is prefixed with a ones column (V' = [1 | V], 65 cols, fp16)
    and pre-arranged into the SBUF image [128(key of block), 16*65]: the PV
    matmul (lhsT = V'[kblk], rhs = exp(S^T)[kblk]) then yields the softmax
    denominator on partition 0 and the numerator on partitions 1-64 in one
    accumulation group. lam is folded into half-2's V on the host, so the
    combine step is a plain subtract.
  - Output stays in [d, q] layout on device; the host transposes it back.

fp16 is used on every matmul path: it streams through the PE at 1
cycle/column (f32 is 4, f32r measured 2) with the same 10-bit mantissa
class as tf32. exp() runs on ScalarE straight out of PSUM at 1 elem/lane/
cycle regardless of dtype, writing fp16.

Main loop per (head, 512-query chunk): 16 key blocks of
  QK matmuls -> exp on ScalarE (PSUM -> SBUF) -> PV accumulate,
then a per-chunk epilogue slice (denominator broadcast on GpSimd, divide +
combine + bn_stats on DVE) that hides under later chunks' main loop.
rstd = Quake-rsqrt on DVE (bitcast + Newton) so ScalarE stays on one
activation table (exp) for the whole kernel.

ScalarE is the bottleneck engine (~2*S^2 exps per head); everything else
is shaped to hide beneath it. A short warm-up matmul spinner at kernel
start flips the PE HAM clock gate to 8/8 before the real matmuls begin.
"""

import math

import numpy as np

B, H, S, D = 1, 16, 2048, 64
N_CORES = 8
HPC = H // N_CORES  # heads per core
QC = 512            # query-chunk width (PSUM bank budget)
N_QC = S // QC
KB = S // 128       # key blocks of 128
LAMBDA_INIT = 0.8
EPS = 1e-5
SCALE = 1.0 / math.sqrt(D)
N_WARMUP_MM = 24

_CACHE = {}


def _build_nc():
    from contextlib import ExitStack

    import concourse.bacc as bacc
    import concourse.bass as bass
    import concourse.tile as tile
    from concourse import bass_isa, mybir

    f32 = mybir.dt.float32
    f16 = mybir.dt.float16
    i32 = mybir.dt.int32
    AF = mybir.ActivationFunctionType
    OP = mybir.AluOpType
    ts = bass.ts

    nc = bacc.Bacc("TRN2", target_bir_lowering=False, debug=False)

    qT = nc.dram_tensor("qT", [HPC, 128, S], f16, kind="ExternalInput").ap()
    kT = nc.dram_tensor("kT", [HPC, 128, S], f16, kind="ExternalInput").ap()
    vp = nc.dram_tensor("vp", [HPC, 2, 128, KB * 65], f16, kind="ExternalInput").ap()
    gb = nc.dram_tensor("gb", [HPC, 64, 2], f32, kind="ExternalInput").ap()
    outT = nc.dram_tensor("outT", [HPC, 64, S], f32, kind="ExternalOutput").ap()

    with tile.TileContext(nc) as tc, ExitStack() as ctx:
        pq = ctx.enter_context(tc.tile_pool(name="pq", bufs=2))
        pk = ctx.enter_context(tc.tile_pool(name="pk", bufs=2))
        pv = ctx.enter_context(tc.tile_pool(name="pv", bufs=2))
        pe = ctx.enter_context(tc.tile_pool(name="pe", bufs=3))
        psa = ctx.enter_context(tc.tile_pool(name="psa", bufs=2))
        pep = ctx.enter_context(tc.tile_pool(name="pep", bufs=2))
        pout = ctx.enter_context(tc.tile_pool(name="pout", bufs=2))
        pst = ctx.enter_context(tc.tile_pool(name="pst", bufs=2))
        psingle = ctx.enter_context(tc.tile_pool(name="psingle", bufs=1))
        psc = ctx.enter_context(tc.tile_pool(name="psc", bufs=2, space="PSUM"))
        pacc = ctx.enter_context(tc.tile_pool(name="pacc", bufs=2, space="PSUM"))

        eps_t = psingle.tile([65, 1], f32)
        nc.vector.memset(eps_t, EPS)
        magic = psingle.tile([65, 1], i32)
        nc.vector.memset(magic, 0x5F3759DF)

        # PE warm-up: ~96 tiny back-to-back matmuls (~6us) flip the HAM
        # clock gate to 8/8 while the first head's DMAs are in flight.
        wu_w = psingle.tile([128, 128], f16)
        nc.vector.memset(wu_w, 0.0)
        wu_ps = psc.tile([128, 2 * QC], f32, tag="sc")
        for _ in range(N_WARMUP_MM):
            nc.tensor.matmul(
                wu_ps[:, 0:128], lhsT=wu_w[:], rhs=wu_w[:], start=True, stop=True
            )

        for h in range(HPC):
            # split loads so the first chunk's matmuls start as early as
            # possible: K halves (k-blocks 0-7 / 8-15), Q per 512-chunk,
            # V' halves.
            ksh = []
            for j in range(2):
                t = pk.tile([128, S // 2], f16, tag=f"ks{j}")
                nc.sync.dma_start(t[:], kT[h, :, j * (S // 2) : (j + 1) * (S // 2)])
                ksh.append(t)
            qsh = []
            for j in range(N_QC):
                t = pq.tile([128, QC], f16, tag=f"qs{j}")
                nc.sync.dma_start(t[:], qT[h, :, j * QC : (j + 1) * QC])
                qsh.append(t)
            vsh = []
            for half in range(2):
                row = []
                for j in range(2):
                    t = pv.tile([128, KB * 65 // 2], f16, tag=f"v{half}{j}")
                    nc.sync.dma_start(
                        t[:],
                        vp[h, half, :, j * (KB * 65 // 2) : (j + 1) * (KB * 65 // 2)],
                    )
                    row.append(t)
                vsh.append(row)
            gbs = pst.tile([65, 2], f32)
            nc.vector.memset(gbs[0:1, :], 0.0)
            nc.gpsimd.dma_start(gbs[1:65, :], gb[h])

            # [denominator(row 0) | numerator(rows 1-64)] x all queries
            sa1 = psa.tile([65, S], f32)
            sa2 = psa.tile([65, S], f32)
            outc = pout.tile([65, S], f32)
            st = pst.tile([65, 2 * N_QC, 6], f32)

            for qc in range(N_QC):
                a1 = pacc.tile([65, QC], f32)
                a2 = pacc.tile([65, QC], f32)
                for k in range(KB):
                    ksk = ksh[k // 8][:, ts(k % 8, 128)]
                    v1k = vsh[0][k // 8][:, ts(k % 8, 65)]
                    v2k = vsh[1][k // 8][:, ts(k % 8, 65)]
                    qsc = qsh[qc]
                    sc = psc.tile([128, 2 * QC], f32, tag="sc")
                    # scores^T block [key 128, query QC] per half
                    nc.tensor.matmul(
                        sc[:, 0:QC],
                        lhsT=ksk[0:64, :],
                        rhs=qsc[0:64, :],
                        start=True,
                        stop=True,
                    )
                    nc.tensor.matmul(
                        sc[:, QC : 2 * QC],
                        lhsT=ksk[64:128, :],
                        rhs=qsc[64:128, :],
                        start=True,
                        stop=True,
                    )
                    e = pe.tile([128, 2 * QC], f16)
                    nc.scalar.activation(e[:], sc[:], AF.Exp, scale=SCALE)
                    nc.tensor.matmul(
                        a1[:],
                        lhsT=v1k,
                        rhs=e[:, 0:QC],
                        start=(k == 0),
                        stop=(k == KB - 1),
                    )
                    nc.tensor.matmul(
                        a2[:],
                        lhsT=v2k,
                        rhs=e[:, QC : 2 * QC],
                        start=(k == 0),
                        stop=(k == KB - 1),
                    )
                nc.vector.tensor_copy(sa1[:, ts(qc, QC)], a1[:])
                nc.vector.tensor_copy(sa2[:, ts(qc, QC)], a2[:])

                # per-chunk epilogue: normalize both halves, combine, and
                # collect bn stats. Done in 256-col pieces so the GpSimd/DVE
                # chain pipelines (matters for the last chunk, which cannot
                # hide under any later main loop).
                for piece in range(2):
                    off = qc * QC + piece * (QC // 2)
                    w = QC // 2
                    rb1 = pep.tile([65, w], f32, tag="rb1")
                    nc.gpsimd.partition_broadcast(
                        rb1[:], sa1[0:1, off : off + w], channels=65
                    )
                    rb2 = pep.tile([65, w], f32, tag="rb2")
                    nc.gpsimd.partition_broadcast(
                        rb2[:], sa2[0:1, off : off + w], channels=65
                    )
                    nc.vector.reciprocal_approx_fast(rb1[:], rb1[:])
                    nc.vector.reciprocal_approx_fast(rb2[:], rb2[:])
                    t1 = pep.tile([65, w], f32, tag="t1")
                    nc.vector.tensor_mul(t1[:], sa1[:, off : off + w], rb1[:])
                    t2 = pep.tile([65, w], f32, tag="t2")
                    nc.vector.tensor_mul(t2[:], sa2[:, off : off + w], rb2[:])
                    nc.vector.tensor_sub(outc[:, off : off + w], t1[:], t2[:])
                    nc.vector.bn_stats(
                        st[:, 2 * qc + piece, :], outc[:, off : off + w]
                    )

            # ---- head finalize (partition 0 rows are harmless zeros) ----
            mv = pst.tile([65, 2], f32)
            nc.vector.bn_aggr(mv[:], st[:])
            s2 = pst.tile([65, 2], f32)
            nc.vector.tensor_copy(s2[:, 0:1], mv[:, 0:1])
            # E[x^2]_p = var_p + mean_p^2
            nc.vector.tensor_scalar(
                out=s2[:, 1:2],
                in0=mv[:, 0:1],
                scalar1=mv[:, 0:1],
                scalar2=mv[:, 1:2],
                op0=OP.mult,
                op1=OP.add,
            )
            tot = pst.tile([65, 2], f32)
            nc.gpsimd.partition_all_reduce(
                tot[:], s2[:], channels=65, reduce_op=bass_isa.ReduceOp.add
            )
            # tot = sums over partitions of per-partition (mean, E[x^2])
            # over 2048 elements; rows 1-64 carry signal -> /64.
            mu = pst.tile([65, 1], f32)
            nc.vector.tensor_scalar_mul(mu[:], tot[:, 0:1], 1.0 / 64.0)
            mu2 = pst.tile([65, 1], f32)
            # mu2 = mu^2 - eps, so veps = tot1/64 - mu2 = var + eps
            nc.vector.tensor_scalar(
                out=mu2[:],
                in0=mu[:],
                scalar1=mu[:],
                scalar2=eps_t[:],
                op0=OP.mult,
                op1=OP.subtract,
            )
            veps = pst.tile([65, 1], f32)
            nc.vector.tensor_scalar(
                out=veps[:],
                in0=tot[:, 1:2],
                scalar1=1.0 / 64.0,
                scalar2=mu2[:],
                op0=OP.mult,
                op1=OP.subtract,
            )
            # rstd = rsqrt(veps): Quake seed + 3 Newton iterations, all DVE
            # (keeps ScalarE on the exp table for the whole kernel).
            ish = pst.tile([65, 1], i32)
            nc.vector.tensor_scalar(
                out=ish[:],
                in0=veps[:].bitcast(i32),
                scalar1=1,
                scalar2=None,
                op0=OP.logical_shift_right,
            )
            iy = pst.tile([65, 1], i32)
            nc.vector.tensor_sub(iy[:], magic[:], ish[:])
            vh = pst.tile([65, 1], f32)
            nc.vector.tensor_scalar_mul(vh[:], veps[:], -0.5)
            cur = iy[:].bitcast(f32)
            for it in range(2):
                aa = pst.tile([65, 1], f32, tag=f"nr_a{it}")
                nc.vector.tensor_mul(aa[:], cur, cur)
                bb = pst.tile([65, 1], f32, tag=f"nr_b{it}")
                nc.vector.tensor_scalar(
                    out=bb[:], in0=aa[:], scalar1=vh[:], scalar2=1.5,
                    op0=OP.mult, op1=OP.add,
                )
                nxt = pst.tile([65, 1], f32, tag=f"nr_y{it}")
                nc.vector.tensor_tensor(out=nxt[:], in0=bb[:], in1=cur, op=OP.mult)
                cur = nxt[:]
            sg = pst.tile([65, 1], f32)
            nc.vector.tensor_tensor(out=sg[:], in0=cur, in1=gbs[:, 0:1], op=OP.mult)
            tb = pst.tile([65, 1], f32)
            ms = pst.tile([65, 1], f32)
            nc.vector.tensor_scalar(
                out=ms[:], in0=mu[:], scalar1=sg[:], scalar2=None, op0=OP.mult
            )
            nc.vector.tensor_sub(tb[:], gbs[:, 1:2], ms[:])
            outf = pout.tile([65, S], f32)
            for piece in range(2):
                sl = slice(piece * (S // 2), (piece + 1) * (S // 2))
                nc.vector.tensor_scalar(
                    out=outf[:, sl],
                    in0=outc[:, sl],
                    scalar1=sg[:],
                    scalar2=tb[:],
                    op0=OP.mult,
                    op1=OP.add,
                )
                nc.sync.dma_start(outT[h, :, sl], outf[1:65, sl])

    nc.compile()
    return nc


def _get_nc():
    if "nc" not in _CACHE:
        _CACHE["nc"] = _build_nc()
    return _CACHE["nc"]


def _host_prep(q, k, v, lq1, lq2, lk1, lk2, gamma, beta):
    """Build per-core input maps."""
    q = np.asarray(q, dtype=np.float32)
    k = np.asarray(k, dtype=np.float32)
    v = np.asarray(v, dtype=np.float32)
    lam = float(
        np.exp(np.float32(np.dot(lq1, lk1)))
        - np.exp(np.float32(np.dot(lq2, lk2)))
        + LAMBDA_INIT
    )
    g2 = (np.asarray(gamma, np.float32) * (1.0 - LAMBDA_INIT)).reshape(H, D)
    b2 = (np.asarray(beta, np.float32) * (1.0 - LAMBDA_INIT)).reshape(H, D)

    in_maps = []
    for c in range(N_CORES):
        heads = range(c * HPC, (c + 1) * HPC)
        qTa = np.empty((HPC, 128, S), np.float16)
        kTa = np.empty((HPC, 128, S), np.float16)
        vpa = np.empty((HPC, 2, 128, KB * 65), np.float16)
        gba = np.empty((HPC, 64, 2), np.float32)
        for i, hh in enumerate(heads):
            qTa[i] = q[0, hh].T.astype(np.float16)
            kTa[i] = k[0, hh].T.astype(np.float16)
            vh = v[0, hh]  # [S, 64]
            v1 = np.concatenate([np.ones((S, 1), np.float32), vh], axis=1)
            v2 = np.concatenate([np.ones((S, 1), np.float32), lam * vh], axis=1)
            # SBUF image: [partition(key within block), kblock*65 + col]
            vpa[i, 0] = (
                v1.reshape(KB, 128, 65).transpose(1, 0, 2).reshape(128, KB * 65)
            ).astype(np.float16)
            vpa[i, 1] = (
                v2.reshape(KB, 128, 65).transpose(1, 0, 2).reshape(128, KB * 65)
            ).astype(np.float16)
            gba[i, :, 0] = g2[hh]
            gba[i, :, 1] = b2[hh]
        in_maps.append({"qT": qTa, "kT": kTa, "vp": vpa, "gb": gba})
    return in_maps


def kernel(q, k, v, lq1, lq2, lk1, lk2, gamma, beta, _trace=False, _tmpdir=None):
    from concourse.bass_utils import run_bass_kernel_spmd

    nc = _get_nc()
    in_maps = _host_prep(q, k, v, lq1, lq2, lk1, lk2, gamma, beta)
    res = run_bass_kernel_spmd(
        nc,
        in_maps,
        core_ids=list(range(N_CORES)),
        trace=_trace,
        tmpdir=_tmpdir,
    )
    out = np.empty((B, H, S, D), np.float32)
    for c in range(N_CORES):
        outT = res.results[c]["outT"]  # [HPC, 64, S]
        for i in range(HPC):
            out[0, c * HPC + i] = outT[i].T
    if _trace:
        _CACHE["last_results"] = res
    return out


# revision 17
# speedup vs baseline: 1.0041x; 1.0041x over previous
"""Differential attention (two-softmax diff + GroupNorm) on 8 TRN2 cores.

Sharding: 16 heads / 8 cores = 2 heads per core (head-parallel, no
collectives). GroupNorm stats are per-(batch, head) so each core is fully
independent.

Device layout choices (host prepares everything):
  - Q, K per head are host-transposed to [128(d), 2048(s)] fp16: partitions
    0-63 hold half-1 (q1/k1), partitions 64-127 hold half-2. QK^T then
    contracts over the partition dim directly, producing transposed score
    blocks S^T[key, query] in PSUM (fp32).
  - V per head is prefixed with a ones column (V' = [1 | V], 65 cols, fp16)
    and pre-arranged into the SBUF image [128(key of block), 16*65]: the PV
    matmul (lhsT = V'[kblk], rhs = exp(S^T)[kblk]) then yields the softmax
    denominator on partition 0 and the numerator on partitions 1-64 in one
    accumulation group. lam is folded into half-2's V on the host, so the
    combine step is a plain subtract.
  - Output stays in [d, q] layout on device; the host transposes it back.

fp16 is used on every matmul path: it streams through the PE at 1
cycle/column (f32 is 4, f32r measured 2) with the same 10-bit mantissa
class as tf32. exp() runs on ScalarE straight out of PSUM at 1 elem/lane/
cycle regardless of dtype, writing fp16.

Main loop per (head, 512-query chunk): 16 key blocks of
  QK matmuls -> exp on ScalarE (PSUM -> SBUF) -> PV accumulate,
then a per-chunk epilogue slice (denominator broadcast on GpSimd, divide +
combine + bn_stats on DVE) that hides under later chunks' main loop.
rstd = Quake-rsqrt on DVE (bitcast + Newton) so ScalarE stays on one
activation table (exp) for the whole kernel.

ScalarE is the bottleneck engine (~2*S^2 exps per head); everything else
is shaped to hide beneath it. A short warm-up matmul spinner at kernel
start flips the PE HAM clock gate to 8/8 before the real matmuls begin.
"""

import math

import numpy as np

B, H, S, D = 1, 16, 2048, 64
N_CORES = 8
HPC = H // N_CORES  # heads per core
QC = 512            # query-chunk width (PSUM bank budget)
N_QC = S // QC
KB = S // 128       # key blocks of 128
LAMBDA_INIT = 0.8
EPS = 1e-5
SCALE = 1.0 / math.sqrt(D)
N_WARMUP_MM = 24

_CACHE = {}


def _build_nc():
    from contextlib import ExitStack

    import concourse.bacc as bacc
    import concourse.bass as bass
    import concourse.tile as tile
    from concourse import bass_isa, mybir

    f32 = mybir.dt.float32
    f16 = mybir.dt.float16
    i32 = mybir.dt.int32
    AF = mybir.ActivationFunctionType
    OP = mybir.AluOpType
    ts = bass.ts

    nc = bacc.Bacc("TRN2", target_bir_lowering=False, debug=False)

    qT = nc.dram_tensor("qT", [HPC, 128, S], f16, kind="ExternalInput").ap()
    kT = nc.dram_tensor("kT", [HPC, 128, S], f16, kind="ExternalInput").ap()
    vp = nc.dram_tensor("vp", [HPC, 2, 128, KB * 65], f16, kind="ExternalInput").ap()
    gb = nc.dram_tensor("gb", [HPC, 64, 2], f32, kind="ExternalInput").ap()
    outT = nc.dram_tensor("outT", [HPC, 64, S], f32, kind="ExternalOutput").ap()

    with tile.TileContext(nc) as tc, ExitStack() as ctx:
        pq = ctx.enter_context(tc.tile_pool(name="pq", bufs=2))
        pk = ctx.enter_context(tc.tile_pool(name="pk", bufs=2))
        pv = ctx.enter_context(tc.tile_pool(name="pv", bufs=2))
        pe = ctx.enter_context(tc.tile_pool(name="pe", bufs=3))
        psa = ctx.enter_context(tc.tile_pool(name="psa", bufs=2))
        pep = ctx.enter_context(tc.tile_pool(name="pep", bufs=2))
        pout = ctx.enter_context(tc.tile_pool(name="pout", bufs=2))
        pst = ctx.enter_context(tc.tile_pool(name="pst", bufs=2))
        psingle = ctx.enter_context(tc.tile_pool(name="psingle", bufs=1))
        psc = ctx.enter_context(tc.tile_pool(name="psc", bufs=2, space="PSUM"))
        pacc = ctx.enter_context(tc.tile_pool(name="pacc", bufs=2, space="PSUM"))

        eps_t = psingle.tile([65, 1], f32)
        nc.vector.memset(eps_t, EPS)
        magic = psingle.tile([65, 1], i32)
        nc.vector.memset(magic, 0x5F3759DF)

        # PE warm-up: ~96 tiny back-to-back matmuls (~6us) flip the HAM
        # clock gate to 8/8 while the first head's DMAs are in flight.
        wu_w = psingle.tile([128, 128], f16)
        nc.vector.memset(wu_w, 0.0)
        wu_ps = psc.tile([128, 2 * QC], f32, tag="sc")
        for _ in range(N_WARMUP_MM):
            nc.tensor.matmul(
                wu_ps[:, 0:128], lhsT=wu_w[:], rhs=wu_w[:], start=True, stop=True
            )

        for h in range(HPC):
            # split loads so the first chunk's matmuls start as early as
            # possible: K halves (k-blocks 0-7 / 8-15), Q per 512-chunk,
            # V' halves.
            ksh = []
            for j in range(2):
                t = pk.tile([128, S // 2], f16, tag=f"ks{j}")
                nc.sync.dma_start(t[:], kT[h, :, j * (S // 2) : (j + 1) * (S // 2)])
                ksh.append(t)
            qsh = []
            for j in range(N_QC):
                t = pq.tile([128, QC], f16, tag=f"qs{j}")
                nc.sync.dma_start(t[:], qT[h, :, j * QC : (j + 1) * QC])
                qsh.append(t)
            vsh = []
            for half in range(2):
                row = []
                for j in range(2):
                    t = pv.tile([128, KB * 65 // 2], f16, tag=f"v{half}{j}")
                    nc.sync.dma_start(
                        t[:],
                        vp[h, half, :, j * (KB * 65 // 2) : (j + 1) * (KB * 65 // 2)],
                    )
                    row.append(t)
                vsh.append(row)
            gbs = pst.tile([65, 2], f32)
            nc.vector.memset(gbs[0:1, :], 0.0)
            nc.gpsimd.dma_start(gbs[1:65, :], gb[h])

            # [denominator(row 0) | numerator(rows 1-64)] x all queries
            sa1 = psa.tile([65, S], f32)
            sa2 = psa.tile([65, S], f32)
            outc = pout.tile([65, S], f32)
            st = pst.tile([65, N_QC, 6], f32)

            for qc in range(N_QC):
                a1 = pacc.tile([65, QC], f32)
                a2 = pacc.tile([65, QC], f32)
                for k in range(KB):
                    ksk = ksh[k // 8][:, ts(k % 8, 128)]
                    v1k = vsh[0][k // 8][:, ts(k % 8, 65)]
                    v2k = vsh[1][k // 8][:, ts(k % 8, 65)]
                    qsc = qsh[qc]
                    sc = psc.tile([128, 2 * QC], f32, tag="sc")
                    # scores^T block [key 128, query QC] per half
                    nc.tensor.matmul(
                        sc[:, 0:QC],
                        lhsT=ksk[0:64, :],
                        rhs=qsc[0:64, :],
                        start=True,
                        stop=True,
                    )
                    nc.tensor.matmul(
                        sc[:, QC : 2 * QC],
                        lhsT=ksk[64:128, :],
                        rhs=qsc[64:128, :],
                        start=True,
                        stop=True,
                    )
                    e = pe.tile([128, 2 * QC], f16)
                    nc.scalar.activation(e[:], sc[:], AF.Exp, scale=SCALE)
                    nc.tensor.matmul(
                        a1[:],
                        lhsT=v1k,
                        rhs=e[:, 0:QC],
                        start=(k == 0),
                        stop=(k == KB - 1),
                    )
                    nc.tensor.matmul(
                        a2[:],
                        lhsT=v2k,
                        rhs=e[:, QC : 2 * QC],
                        start=(k == 0),
                        stop=(k == KB - 1),
                    )
                nc.vector.tensor_copy(sa1[:, ts(qc, QC)], a1[:])
                nc.vector.tensor_copy(sa2[:, ts(qc, QC)], a2[:])

                # per-chunk epilogue slice: normalize both halves, combine,
                # and collect bn stats; hides under later chunks' main loop.
                rb1 = pep.tile([65, QC], f32)
                nc.gpsimd.partition_broadcast(
                    rb1[:], sa1[0:1, ts(qc, QC)], channels=65
                )
                rb2 = pep.tile([65, QC], f32)
                nc.gpsimd.partition_broadcast(
                    rb2[:], sa2[0:1, ts(qc, QC)], channels=65
                )
                nc.vector.reciprocal_approx_fast(rb1[:], rb1[:])
                nc.vector.reciprocal_approx_fast(rb2[:], rb2[:])
                t1 = pep.tile([65, QC], f32)
                nc.vector.tensor_mul(t1[:], sa1[:, ts(qc, QC)], rb1[:])
                t2 = pep.tile([65, QC], f32)
                nc.vector.tensor_mul(t2[:], sa2[:, ts(qc, QC)], rb2[:])
                nc.vector.tensor_sub(outc[:, ts(qc, QC)], t1[:], t2[:])
                nc.vector.bn_stats(st[:, qc, :], outc[:, ts(qc, QC)])

            # ---- head finalize (partition 0 rows are harmless zeros) ----
            mv = pst.tile([65, 2], f32)
            nc.vector.bn_aggr(mv[:], st[:])
            s2 = pst.tile([65, 2], f32)
            nc.vector.tensor_copy(s2[:, 0:1], mv[:, 0:1])
            # E[x^2]_p = var_p + mean_p^2
            nc.vector.tensor_scalar(
                out=s2[:, 1:2],
                in0=mv[:, 0:1],
                scalar1=mv[:, 0:1],
                scalar2=mv[:, 1:2],
                op0=OP.mult,
                op1=OP.add,
            )
            tot = pst.tile([65, 2], f32)
            nc.gpsimd.partition_all_reduce(
                tot[:], s2[:], channels=65, reduce_op=bass_isa.ReduceOp.add
            )
            # tot = sums over partitions of per-partition (mean, E[x^2])
            # over 2048 elements; rows 1-64 carry signal -> /64.
            mu = pst.tile([65, 1], f32)
            nc.vector.tensor_scalar_mul(mu[:], tot[:, 0:1], 1.0 / 64.0)
            mu2 = pst.tile([65, 1], f32)
            # mu2 = mu^2 - eps, so veps = tot1/64 - mu2 = var + eps
            nc.vector.tensor_scalar(
                out=mu2[:],
                in0=mu[:],
                scalar1=mu[:],
                scalar2=eps_t[:],
                op0=OP.mult,
                op1=OP.subtract,
            )
            veps = pst.tile([65, 1], f32)
            nc.vector.tensor_scalar(
                out=veps[:],
                in0=tot[:, 1:2],
                scalar1=1.0 / 64.0,
                scalar2=mu2[:],
                op0=OP.mult,
                op1=OP.subtract,
            )
            # rstd = rsqrt(veps): Quake seed + 3 Newton iterations, all DVE
            # (keeps ScalarE on the exp table for the whole kernel).
            ish = pst.tile([65, 1], i32)
            nc.vector.tensor_scalar(
                out=ish[:],
                in0=veps[:].bitcast(i32),
                scalar1=1,
                scalar2=None,
                op0=OP.logical_shift_right,
            )
            iy = pst.tile([65, 1], i32)
            nc.vector.tensor_sub(iy[:], magic[:], ish[:])
            vh = pst.tile([65, 1], f32)
            nc.vector.tensor_scalar_mul(vh[:], veps[:], -0.5)
            cur = iy[:].bitcast(f32)
            for it in range(2):
                aa = pst.tile([65, 1], f32, tag=f"nr_a{it}")
                nc.vector.tensor_mul(aa[:], cur, cur)
                bb = pst.tile([65, 1], f32, tag=f"nr_b{it}")
                nc.vector.tensor_scalar(
                    out=bb[:], in0=aa[:], scalar1=vh[:], scalar2=1.5,
                    op0=OP.mult, op1=OP.add,
                )
                nxt = pst.tile([65, 1], f32, tag=f"nr_y{it}")
                nc.vector.tensor_tensor(out=nxt[:], in0=bb[:], in1=cur, op=OP.mult)
                cur = nxt[:]
            sg = pst.tile([65, 1], f32)
            nc.vector.tensor_tensor(out=sg[:], in0=cur, in1=gbs[:, 0:1], op=OP.mult)
            tb = pst.tile([65, 1], f32)
            ms = pst.tile([65, 1], f32)
            nc.vector.tensor_scalar(
                out=ms[:], in0=mu[:], scalar1=sg[:], scalar2=None, op0=OP.mult
            )
            nc.vector.tensor_sub(tb[:], gbs[:, 1:2], ms[:])
            outf = pout.tile([65, S], f32)
            for piece in range(2):
                sl = slice(piece * (S // 2), (piece + 1) * (S // 2))
                nc.vector.tensor_scalar(
                    out=outf[:, sl],
                    in0=outc[:, sl],
                    scalar1=sg[:],
                    scalar2=tb[:],
                    op0=OP.mult,
                    op1=OP.add,
                )
                nc.sync.dma_start(outT[h, :, sl], outf[1:65, sl])

    nc.compile()
    return nc


def _get_nc():
    if "nc" not in _CACHE:
        _CACHE["nc"] = _build_nc()
    return _CACHE["nc"]


def _host_prep(q, k, v, lq1, lq2, lk1, lk2, gamma, beta):
    """Build per-core input maps."""
    q = np.asarray(q, dtype=np.float32)
    k = np.asarray(k, dtype=np.float32)
    v = np.asarray(v, dtype=np.float32)
    lam = float(
        np.exp(np.float32(np.dot(lq1, lk1)))
        - np.exp(np.float32(np.dot(lq2, lk2)))
        + LAMBDA_INIT
    )
    g2 = (np.asarray(gamma, np.float32) * (1.0 - LAMBDA_INIT)).reshape(H, D)
    b2 = (np.asarray(beta, np.float32) * (1.0 - LAMBDA_INIT)).reshape(H, D)

    in_maps = []
    for c in range(N_CORES):
        heads = range(c * HPC, (c + 1) * HPC)
        qTa = np.empty((HPC, 128, S), np.float16)
        kTa = np.empty((HPC, 128, S), np.float16)
        vpa = np.empty((HPC, 2, 128, KB * 65), np.float16)
        gba = np.empty((HPC, 64, 2), np.float32)
        for i, hh in enumerate(heads):
            qTa[i] = q[0, hh].T.astype(np.float16)
            kTa[i] = k[0, hh].T.astype(np.float16)
            vh = v[0, hh]  # [S, 64]
            v1 = np.concatenate([np.ones((S, 1), np.float32), vh], axis=1)
            v2 = np.concatenate([np.ones((S, 1), np.float32), lam * vh], axis=1)
            # SBUF image: [partition(key within block), kblock*65 + col]
            vpa[i, 0] = (
                v1.reshape(KB, 128, 65).transpose(1, 0, 2).reshape(128, KB * 65)
            ).astype(np.float16)
            vpa[i, 1] = (
                v2.reshape(KB, 128, 65).transpose(1, 0, 2).reshape(128, KB * 65)
            ).astype(np.float16)
            gba[i, :, 0] = g2[hh]
            gba[i, :, 1] = b2[hh]
        in_maps.append({"qT": qTa, "kT": kTa, "vp": vpa, "gb": gba})
    return in_maps


def kernel(q, k, v, lq1, lq2, lk1, lk2, gamma, beta, _trace=False, _tmpdir=None):
    from concourse.bass_utils import run_bass_kernel_spmd

    nc = _get_nc()
    in_maps = _host_prep(q, k, v, lq1, lq2, lk1, lk2, gamma, beta)
    res = run_bass_kernel_spmd(
        nc,
        in_maps,
        core_ids=list(range(N_CORES)),
        trace=_trace,
        tmpdir=_tmpdir,
    )
    out = np.empty((B, H, S, D), np.float32)
    for c in range(N_CORES):
        outT = res.results[c]["outT"]  # [HPC, 64, S]
        for i in range(HPC):
            out[0, c * HPC + i] = outT[i].T
    if _trace:
        _CACHE["last_results"] = res
    return out


# revision 18
# speedup vs baseline: 1.1923x; 1.1874x over previous
"""Differential attention (two-softmax diff + GroupNorm) on 8 TRN2 cores.

Sharding: 16 heads / 8 cores = 2 heads per core (head-parallel, no
collectives). GroupNorm stats are per-(batch, head) so each core is fully
independent.

Device layout choices (host prepares everything):
  - Q, K per head are host-transposed to [128(d), 2048(s)] fp16: partitions
    0-63 hold half-1 (q1/k1), partitions 64-127 hold half-2. QK^T then
    contracts over the partition dim directly, producing transposed score
    blocks S^T[key, query] in PSUM (fp32).
  - V per head is prefixed with a ones column (V' = [1 | V], 65 cols, fp16)
    and pre-arranged into the SBUF image [128(key of block), 16*65]: the PV
    matmul (lhsT = V'[kblk], rhs = exp(S^T)[kblk]) then yields the softmax
    denominator on partition 0 and the numerator on partitions 1-64 in one
    accumulation group. lam is folded into half-2's V on the host, so the
    combine step is a plain subtract.
  - Output stays in [d, q] layout on device; the host transposes it back.

fp16 is used on every matmul path: it streams through the PE at 1
cycle/column (f32 is 4, f32r measured 2) with the same 10-bit mantissa
class as tf32. exp() runs on ScalarE straight out of PSUM at 1 elem/lane/
cycle regardless of dtype, writing fp16.

Main loop per (head, 512-query chunk): 16 key blocks of
  QK matmuls -> exp on ScalarE (PSUM -> SBUF) -> PV accumulate,
then a per-chunk epilogue slice (denominator broadcast on GpSimd, divide +
combine + bn_stats on DVE) that hides under later chunks' main loop.
rstd = Quake-rsqrt on DVE (bitcast + Newton) so ScalarE stays on one
activation table (exp) for the whole kernel.

ScalarE is the bottleneck engine (~2*S^2 exps per head); everything else
is shaped to hide beneath it. A short warm-up matmul spinner at kernel
start flips the PE HAM clock gate to 8/8 before the real matmuls begin.
"""

import math

import numpy as np

B, H, S, D = 1, 16, 2048, 64
N_CORES = 8
HPC = H // N_CORES  # heads per core
QC = 512            # query-chunk width (PSUM bank budget)
N_QC = S // QC
KB = S // 128       # key blocks of 128
LAMBDA_INIT = 0.8
EPS = 1e-5
SCALE = 1.0 / math.sqrt(D)
N_WARMUP_MM = 24

_CACHE = {}


def _build_nc():
    from contextlib import ExitStack

    import concourse.bacc as bacc
    import concourse.bass as bass
    import concourse.tile as tile
    from concourse import bass_isa, mybir

    f32 = mybir.dt.float32
    f16 = mybir.dt.float16
    i32 = mybir.dt.int32
    AF = mybir.ActivationFunctionType
    OP = mybir.AluOpType
    ts = bass.ts

    nc = bacc.Bacc("TRN2", target_bir_lowering=False, debug=False)

    qT = nc.dram_tensor("qT", [HPC, 128, S], f16, kind="ExternalInput").ap()
    kT = nc.dram_tensor("kT", [HPC, 128, S], f16, kind="ExternalInput").ap()
    vp = nc.dram_tensor("vp", [HPC, 2, 128, KB * 65], f16, kind="ExternalInput").ap()
    gb = nc.dram_tensor("gb", [HPC, 64, 2], f32, kind="ExternalInput").ap()
    outT = nc.dram_tensor("outT", [HPC, 64, S], f32, kind="ExternalOutput").ap()

    with tile.TileContext(nc) as tc, ExitStack() as ctx:
        pq = ctx.enter_context(tc.tile_pool(name="pq", bufs=2))
        pk = ctx.enter_context(tc.tile_pool(name="pk", bufs=2))
        pv = ctx.enter_context(tc.tile_pool(name="pv", bufs=2))
        pe = ctx.enter_context(tc.tile_pool(name="pe", bufs=3))
        psa = ctx.enter_context(tc.tile_pool(name="psa", bufs=2))
        pep = ctx.enter_context(tc.tile_pool(name="pep", bufs=2))
        pout = ctx.enter_context(tc.tile_pool(name="pout", bufs=2))
        pst = ctx.enter_context(tc.tile_pool(name="pst", bufs=2))
        psingle = ctx.enter_context(tc.tile_pool(name="psingle", bufs=1))
        psc = ctx.enter_context(tc.tile_pool(name="psc", bufs=2, space="PSUM"))
        pacc = ctx.enter_context(tc.tile_pool(name="pacc", bufs=2, space="PSUM"))

        eps_t = psingle.tile([65, 1], f32)
        nc.vector.memset(eps_t, EPS)
        magic = psingle.tile([65, 1], i32)
        nc.vector.memset(magic, 0x5F3759DF)

        # PE warm-up: ~96 tiny back-to-back matmuls (~6us) flip the HAM
        # clock gate to 8/8 while the first head's DMAs are in flight.
        wu_w = psingle.tile([128, 128], f16)
        nc.vector.memset(wu_w, 0.0)
        wu_ps = psc.tile([128, 2 * QC], f32, tag="sc")
        for _ in range(N_WARMUP_MM):
            nc.tensor.matmul(
                wu_ps[:, 0:128], lhsT=wu_w[:], rhs=wu_w[:], start=True, stop=True
            )

        for h in range(HPC):
            # split loads so the first chunk's matmuls start as early as
            # possible: K halves (k-blocks 0-7 / 8-15), Q per 512-chunk,
            # V' halves.
            ksh = []
            for j in range(2):
                t = pk.tile([128, S // 2], f16, tag=f"ks{j}")
                nc.sync.dma_start(t[:], kT[h, :, j * (S // 2) : (j + 1) * (S // 2)])
                ksh.append(t)
            qsh = []
            for j in range(N_QC):
                t = pq.tile([128, QC], f16, tag=f"qs{j}")
                nc.sync.dma_start(t[:], qT[h, :, j * QC : (j + 1) * QC])
                qsh.append(t)
            vsh = []
            for half in range(2):
                row = []
                for j in range(2):
                    t = pv.tile([128, KB * 65 // 2], f16, tag=f"v{half}{j}")
                    nc.sync.dma_start(
                        t[:],
                        vp[h, half, :, j * (KB * 65 // 2) : (j + 1) * (KB * 65 // 2)],
                    )
                    row.append(t)
                vsh.append(row)
            gbs = pst.tile([65, 2], f32)
            nc.vector.memset(gbs[0:1, :], 0.0)
            nc.gpsimd.dma_start(gbs[1:65, :], gb[h])

            # [denominator(row 0) | numerator(rows 1-64)] x all queries
            sa1 = psa.tile([65, S], f32)
            sa2 = psa.tile([65, S], f32)
            outc = pout.tile([65, S], f32)
            st = pst.tile([65, N_QC, 6], f32)

            for qc in range(N_QC):
                a1 = pacc.tile([65, QC], f32)
                a2 = pacc.tile([65, QC], f32)
                for k in range(KB):
                    ksk = ksh[k // 8][:, ts(k % 8, 128)]
                    v1k = vsh[0][k // 8][:, ts(k % 8, 65)]
                    v2k = vsh[1][k // 8][:, ts(k % 8, 65)]
                    qsc = qsh[qc]
                    sc = psc.tile([128, 2 * QC], f32, tag="sc")
                    # scores^T block [key 128, query QC] per half
                    nc.tensor.matmul(
                        sc[:, 0:QC],
                        lhsT=ksk[0:64, :],
                        rhs=qsc[0:64, :],
                        start=True,
                        stop=True,
                    )
                    nc.tensor.matmul(
                        sc[:, QC : 2 * QC],
                        lhsT=ksk[64:128, :],
                        rhs=qsc[64:128, :],
                        start=True,
                        stop=True,
                    )
                    e = pe.tile([128, 2 * QC], f16)
                    nc.scalar.activation(e[:], sc[:], AF.Exp, scale=SCALE)
                    nc.tensor.matmul(
                        a1[:],
                        lhsT=v1k,
                        rhs=e[:, 0:QC],
                        start=(k == 0),
                        stop=(k == KB - 1),
                    )
                    nc.tensor.matmul(
                        a2[:],
                        lhsT=v2k,
                        rhs=e[:, QC : 2 * QC],
                        start=(k == 0),
                        stop=(k == KB - 1),
                    )
                nc.vector.tensor_copy(sa1[:, ts(qc, QC)], a1[:])
                nc.vector.tensor_copy(sa2[:, ts(qc, QC)], a2[:])

                # per-chunk epilogue slice: normalize both halves, combine,
                # and collect bn stats; hides under later chunks' main loop.
                rb1 = pep.tile([65, QC], f32)
                nc.gpsimd.partition_broadcast(
                    rb1[:], sa1[0:1, ts(qc, QC)], channels=65
                )
                rb2 = pep.tile([65, QC], f32)
                nc.gpsimd.partition_broadcast(
                    rb2[:], sa2[0:1, ts(qc, QC)], channels=65
                )
                nc.vector.reciprocal_approx_fast(rb1[:], rb1[:])
                nc.vector.reciprocal_approx_fast(rb2[:], rb2[:])
                t1 = pep.tile([65, QC], f32)
                nc.vector.tensor_mul(t1[:], sa1[:, ts(qc, QC)], rb1[:])
                t2 = pep.tile([65, QC], f32)
                nc.vector.tensor_mul(t2[:], sa2[:, ts(qc, QC)], rb2[:])
                nc.vector.tensor_sub(outc[:, ts(qc, QC)], t1[:], t2[:])
                nc.vector.bn_stats(st[:, qc, :], outc[:, ts(qc, QC)])

            # ---- head finalize (partition 0 rows are harmless zeros) ----
            mv = pst.tile([65, 2], f32)
            nc.vector.bn_aggr(mv[:], st[:])
            s2 = pst.tile([65, 2], f32)
            nc.vector.tensor_copy(s2[:, 0:1], mv[:, 0:1])
            # E[x^2]_p = var_p + mean_p^2
            nc.vector.tensor_scalar(
                out=s2[:, 1:2],
                in0=mv[:, 0:1],
                scalar1=mv[:, 0:1],
                scalar2=mv[:, 1:2],
                op0=OP.mult,
                op1=OP.add,
            )
            tot = pst.tile([65, 2], f32)
            nc.gpsimd.partition_all_reduce(
                tot[:], s2[:], channels=65, reduce_op=bass_isa.ReduceOp.add
            )
            # tot = sums over partitions of per-partition (mean, E[x^2])
            # over 2048 elements; rows 1-64 carry signal -> /64.
            mu = pst.tile([65, 1], f32)
            nc.vector.tensor_scalar_mul(mu[:], tot[:, 0:1], 1.0 / 64.0)
            mu2 = pst.tile([65, 1], f32)
            # mu2 = mu^2 - eps, so veps = tot1/64 - mu2 = var + eps
            nc.vector.tensor_scalar(
                out=mu2[:],
                in0=mu[:],
                scalar1=mu[:],
                scalar2=eps_t[:],
                op0=OP.mult,
                op1=OP.subtract,
            )
            veps = pst.tile([65, 1], f32)
            nc.vector.tensor_scalar(
                out=veps[:],
                in0=tot[:, 1:2],
                scalar1=1.0 / 64.0,
                scalar2=mu2[:],
                op0=OP.mult,
                op1=OP.subtract,
            )
            # rstd = rsqrt(veps): Quake seed + 3 Newton iterations, all DVE
            # (keeps ScalarE on the exp table for the whole kernel).
            ish = pst.tile([65, 1], i32)
            nc.vector.tensor_scalar(
                out=ish[:],
                in0=veps[:].bitcast(i32),
                scalar1=1,
                scalar2=None,
                op0=OP.logical_shift_right,
            )
            iy = pst.tile([65, 1], i32)
            nc.vector.tensor_sub(iy[:], magic[:], ish[:])
            vh = pst.tile([65, 1], f32)
            nc.vector.tensor_scalar_mul(vh[:], veps[:], -0.5)
            cur = iy[:].bitcast(f32)
            for it in range(2):
                aa = pst.tile([65, 1], f32, tag=f"nr_a{it}")
                nc.vector.tensor_mul(aa[:], cur, cur)
                bb = pst.tile([65, 1], f32, tag=f"nr_b{it}")
                nc.vector.tensor_scalar(
                    out=bb[:], in0=aa[:], scalar1=vh[:], scalar2=1.5,
                    op0=OP.mult, op1=OP.add,
                )
                nxt = pst.tile([65, 1], f32, tag=f"nr_y{it}")
                nc.vector.tensor_tensor(out=nxt[:], in0=bb[:], in1=cur, op=OP.mult)
                cur = nxt[:]
            sg = pst.tile([65, 1], f32)
            nc.vector.tensor_tensor(out=sg[:], in0=cur, in1=gbs[:, 0:1], op=OP.mult)
            tb = pst.tile([65, 1], f32)
            ms = pst.tile([65, 1], f32)
            nc.vector.tensor_scalar(
                out=ms[:], in0=mu[:], scalar1=sg[:], scalar2=None, op0=OP.mult
            )
            nc.vector.tensor_sub(tb[:], gbs[:, 1:2], ms[:])
            outf = pout.tile([65, S], f32)
            nc.vector.tensor_scalar(
                out=outf[:],
                in0=outc[:],
                scalar1=sg[:],
                scalar2=tb[:],
                op0=OP.mult,
                op1=OP.add,
            )
            nc.sync.dma_start(outT[h], outf[1:65, :])

    nc.compile()
    return nc


def _get_nc():
    if "nc" not in _CACHE:
        _CACHE["nc"] = _build_nc()
    return _CACHE["nc"]


def _host_prep(q, k, v, lq1, lq2, lk1, lk2, gamma, beta):
    """Build per-core input maps."""
    q = np.asarray(q, dtype=np.float32)
    k = np.asarray(k, dtype=np.float32)
    v = np.asarray(v, dtype=np.float32)
    lam = float(
        np.exp(np.float32(np.dot(lq1, lk1)))
        - np.exp(np.float32(np.dot(lq2, lk2)))
        + LAMBDA_INIT
    )
    g2 = (np.asarray(gamma, np.float32) * (1.0 - LAMBDA_INIT)).reshape(H, D)
    b2 = (np.asarray(beta, np.float32) * (1.0 - LAMBDA_INIT)).reshape(H, D)

    in_maps = []
    for c in range(N_CORES):
        heads = range(c * HPC, (c + 1) * HPC)
        qTa = np.empty((HPC, 128, S), np.float16)
        kTa = np.empty((HPC, 128, S), np.float16)
        vpa = np.empty((HPC, 2, 128, KB * 65), np.float16)
        gba = np.empty((HPC, 64, 2), np.float32)
        for i, hh in enumerate(heads):
            qTa[i] = q[0, hh].T.astype(np.float16)
            kTa[i] = k[0, hh].T.astype(np.float16)
            vh = v[0, hh]  # [S, 64]
            v1 = np.concatenate([np.ones((S, 1), np.float32), vh], axis=1)
            v2 = np.concatenate([np.ones((S, 1), np.float32), lam * vh], axis=1)
            # SBUF image: [partition(key within block), kblock*65 + col]
            vpa[i, 0] = (
                v1.reshape(KB, 128, 65).transpose(1, 0, 2).reshape(128, KB * 65)
            ).astype(np.float16)
            vpa[i, 1] = (
                v2.reshape(KB, 128, 65).transpose(1, 0, 2).reshape(128, KB * 65)
            ).astype(np.float16)
            gba[i, :, 0] = g2[hh]
            gba[i, :, 1] = b2[hh]
        in_maps.append({"qT": qTa, "kT": kTa, "vp": vpa, "gb": gba})
    return in_maps


def kernel(q, k, v, lq1, lq2, lk1, lk2, gamma, beta, _trace=False, _tmpdir=None):
    from concourse.bass_utils import run_bass_kernel_spmd

    nc = _get_nc()
    in_maps = _host_prep(q, k, v, lq1, lq2, lk1, lk2, gamma, beta)
    res = run_bass_kernel_spmd(
        nc,
        in_maps,
        core_ids=list(range(N_CORES)),
        trace=_trace,
        tmpdir=_tmpdir,
    )
    out = np.empty((B, H, S, D), np.float32)
    for c in range(N_CORES):
        outT = res.results[c]["outT"]  # [HPC, 64, S]
        for i in range(HPC):
            out[0, c * HPC + i] = outT[i].T
    if _trace:
        _CACHE["last_results"] = res
    return out
